# revision 16
# baseline (speedup 1.0000x reference)
import sys
sys.path.insert(0, '/opt/trn_rl_repo')
import numpy as np
import ml_dtypes
import concourse.bass as bass
import concourse.bacc as bacc
import concourse.tile as tile
import concourse.mybir as mybir
from concourse.bass_utils import run_bass_kernel_spmd

C3_TABLE = [(0, 1, 2), (1, 2, 3), (2, 3, 4), (3, 4, 5), (0, 4, 5), (0, 1, 5),
            (0, 1, 2, 3), (1, 2, 3, 4), (2, 3, 4, 5), (0, 3, 4, 5), (0, 1, 4, 5),
            (0, 1, 2, 5), (0, 1, 3, 4), (1, 2, 4, 5), (0, 2, 3, 5),
            (0, 1, 2, 3, 4, 5)]
A = 1.7159
S = 2.0 / 3.0
Q = 127.0                      # int8 quant scale for tanh in [-1, 1]

B, C, H, W = 256, 6, 142, 142
KH = KW = 5
OC = 16
OH, OW = H - 4, W - 4          # 138
NCORES = 8
B_LOC = B // NCORES            # 32
BF16 = ml_dtypes.bfloat16

# Column-parity formulation with pass-offset views: output (oc, j, p) at
# stream col (b, u) is y[b, oc, oh0 + j, 2u + p].  x is stored once as two
# column-parity classes; pass g reads the same tile at u-offset g, so its
# rows act as shift classes {2g, 2g+1} covering taps kw = 2g + cls - p.
T = 4                          # output rows per block
SPAN = 8                       # hh window
P2 = 2                         # column parities
U = OW // 2                    # 69 streamed positions
U2 = U + 2                     # 71 stored positions
M = OC * T * P2                # 128: m = oc*8 + j*2 + p
ROWB = 2 * C                   # 12 rows per hslot (cls, c)
KROWS = SPAN * ROWB            # 96
NST = 3                        # interleaved streams (ring prefetch depth)
SB = OH // NST                 # 46 output rows per stream
NBS = 12                       # blocks per stream (11 full + 1 two-row tail)
NSTEP = NST * NBS              # 36
NP = 3                         # passes (kw class pairs)
CB = 4                         # batches per psum chunk
NS = CB * U                    # 276 cols per matmul
PB = 512                       # psum bank stride (fp32)
SF = B_LOC * U                 # 2208 output cols per step
XCOL = B_LOC * U2              # 2272 stored cols per row
XROW = ROWB * XCOL             # 27264 elems per h in DRAM

_cache = {}


def _build():
    if 'nc' in _cache:
        return _cache['nc']
    f32 = mybir.dt.float32
    bf16 = mybir.dt.bfloat16
    i8 = mybir.dt.int8
    nc = bacc.Bacc("TRN2", target_bir_lowering=False, debug=False,
                   num_devices=NCORES)
    x_d = nc.dram_tensor("x", [H, XROW], bf16, kind="ExternalInput").ap()
    w_d = nc.dram_tensor("w", [KROWS, 9 * M], bf16, kind="ExternalInput").ap()
    b_d = nc.dram_tensor("b", [M, 1], f32, kind="ExternalInput").ap()
    y_d = nc.dram_tensor("y", [NSTEP, M, SF], i8, kind="ExternalOutput").ap()

    with tile.TileContext(nc) as tc:
        with tc.tile_pool(name="wpool", bufs=1) as wpool, \
             tc.tile_pool(name="tpool", bufs=3) as tpool, \
             tc.tile_pool(name="spool", bufs=3) as spool, \
             tc.tile_pool(name="pspool", bufs=1, space="PSUM") as pspool:

            xt = [wpool.tile([KROWS, XCOL], bf16, name=f"xt{k}")
                  for k in range(NST)]
            w_sb = wpool.tile([KROWS, 9 * M], bf16)
            b_sb = wpool.tile([M, 1], f32)

            # preload: stream 0's first batch-chunk first so matmuls start
            # early; streams 1/2 ride the scalar HWDGE queue set
            nc.gpsimd.dma_start(w_sb[:], w_d[:])
            nc.gpsimd.dma_start(b_sb[:], b_d[:])
            src0 = x_d[0:SPAN].rearrange("h (r n) -> (h r) n", r=ROWB)
            nc.sync.dma_start(xt[0][:, 0:CB * U2], src0[:, 0:CB * U2])
            nc.sync.dma_start(xt[0][:, CB * U2:], src0[:, CB * U2:])
            for k in (1, 2):
                src = x_d[SB * k:SB * k + SPAN]
                nc.gpsimd.dma_start(
                    xt[k][:], src.rearrange("h (r n) -> (h r) n", r=ROWB))

            for i in range(NBS):
                var = 2 if i == 11 else (i % 2)
                for k in range(NST):
                    step = i * NST + k
                    t_sb = tpool.tile([M, SF], bf16)
                    stage = spool.tile([M, SF], i8)
                    for half in range(2):
                        ps = pspool.tile([M, 4 * PB], f32, name=f"ps{half}",
                                         tag=f"ps{half}")
                        for k4 in range(4):
                            ch = half * 4 + k4
                            for g in range(NP):
                                rv = xt[k][:].rearrange(
                                    "k (b u) -> k b u", b=B_LOC)[
                                    :, ch * CB:(ch + 1) * CB, g:g + U]
                                nc.tensor.matmul(
                                    ps[:, k4 * PB:k4 * PB + NS],
                                    w_sb[:, (var * NP + g) * M:
                                         (var * NP + g + 1) * M],
                                    rv, start=(g == 0), stop=(g == NP - 1))
                        src_v = ps[:].rearrange("m (c n) -> m c n",
                                                n=PB)[:, :, 0:NS]
                        t_sl = t_sb[:, half * 4 * NS:(half + 1) * 4 * NS]
                        nc.scalar.activation(
                            t_sl.rearrange("m (c n) -> m c n", n=NS), src_v,
                            mybir.ActivationFunctionType.Tanh,
                            bias=b_sb[:], scale=S)
                        if half == 1 and i + 1 < NBS:
                            # ring refill for (k, i+1)
                            h0 = SB * k + 4 * (i + 1) + 4
                            hn = min(SB * k + 50, h0 + 4) - h0
                            sl = (4 * (i + 1) + 4) % 8
                            src = x_d[h0:h0 + hn]
                            nc.sync.dma_start(
                                xt[k][sl * ROWB:(sl + hn) * ROWB, :],
                                src.rearrange("h (r n) -> (h r) n", r=ROWB))
                        hs = slice(half * 4 * NS, (half + 1) * 4 * NS)
                        nc.vector.tensor_scalar_mul(stage[:, hs], t_sb[:, hs], Q)
                        nc.gpsimd.dma_start(y_d[step, :, hs], stage[:, hs])
    nc.compile()
    _cache['nc'] = nc
    return nc


def _prep_weights(w3, b3, w4, b4, w6, b6):
    Wd = np.zeros((OC, C, KH, KW), np.float32)
    bias = np.zeros((OC,), np.float32)
    for i, idx in enumerate(C3_TABLE[:6]):
        Wd[i, list(idx)] = w3[i]
        bias[i] = b3[i]
    for i, idx in enumerate(C3_TABLE[6:15]):
        Wd[6 + i, list(idx)] = w4[i]
        bias[6 + i] = b4[i]
    Wd[15, list(C3_TABLE[15])] = w6[0]
    bias[15] = b6[0]

    def build(rot, jmax):
        w = np.zeros((NP, KROWS, M), np.float32)
        for g in range(NP):
            for slot in range(SPAN):
                hh = (slot - rot) % 8
                for j in range(jmax):
                    kh = hh - j
                    if not (0 <= kh < KH):
                        continue
                    for cls in range(2):
                        for p in range(P2):
                            kw = 2 * g + cls - p
                            if not (0 <= kw < KW):
                                continue
                            k0 = slot * ROWB + cls * C
                            w[g, k0:k0 + C,
                              np.arange(OC) * 8 + j * 2 + p] = Wd[:, :, kh, kw]
        return w

    wk = np.stack([build(0, 4), build(4, 4), build(4, 2)])  # [var, g, k, m]
    wflat = wk.transpose(2, 0, 1, 3).reshape(KROWS, 9 * M)
    bvec = np.repeat(S * bias, T * P2).reshape(M, 1).astype(np.float32)
    return wflat.astype(BF16), bvec


def _prep_x(x_shard):
    # [B_LOC, C, H, W] f32 -> [H, (cls c b u)] bf16 parity classes
    xc = np.empty((H, 2, C, B_LOC, U2), BF16)
    for cls in range(2):
        xc[:, cls] = x_shard[:, :, :, cls:cls + 2 * U2:2].transpose(2, 1, 0, 3)
    return np.ascontiguousarray(xc.reshape(H, XROW))


def _unpack_y(y_s):
    v = np.asarray(y_s).astype(np.float32)
    v = v.reshape(NBS, NST, OC, T, P2, B_LOC, U)    # i, k, oc, j, p, b, u
    out = np.empty((B_LOC, OC, OH, OW), np.float32)
    for i in range(NBS):
        jmax = 2 if i == 11 else 4
        for k in range(NST):
            oh0 = SB * k + 4 * i
            blk = v[i, k, :, :jmax]                 # oc, j, p, b, u
            out[:, :, oh0:oh0 + jmax, 0::2] = blk[:, :, 0].transpose(2, 0, 1, 3)
            out[:, :, oh0:oh0 + jmax, 1::2] = blk[:, :, 1].transpose(2, 0, 1, 3)
    return out


def kernel(x, w3, b3, w4, b4, w6, b6):
    nc = _build()
    w3, b3, w4, b4, w6, b6 = [np.asarray(a, dtype=np.float32)
                              for a in (w3, b3, w4, b4, w6, b6)]
    wk, bvec = _prep_weights(w3, b3, w4, b4, w6, b6)
    x = np.ascontiguousarray(np.asarray(x), dtype=np.float32)
    in_maps = [{"x": _prep_x(x[i * B_LOC:(i + 1) * B_LOC]), "w": wk, "b": bvec}
               for i in range(NCORES)]
    res = run_bass_kernel_spmd(nc, in_maps, list(range(NCORES)))
    out = np.concatenate([_unpack_y(res.results[i]["y"]) for i in range(NCORES)],
                         axis=0)
    out *= (A / Q)
    return np.ascontiguousarray(out)


# revision 17
# speedup vs baseline: 1.0305x; 1.0305x over previous
import sys
sys.path.insert(0, '/opt/trn_rl_repo')
import numpy as np
import ml_dtypes
import concourse.bass as bass
import concourse.bacc as bacc
import concourse.tile as tile
import concourse.mybir as mybir
from concourse.bass_utils import run_bass_kernel_spmd

C3_TABLE = [(0, 1, 2), (1, 2, 3), (2, 3, 4), (3, 4, 5), (0, 4, 5), (0, 1, 5),
            (0, 1, 2, 3), (1, 2, 3, 4), (2, 3, 4, 5), (0, 3, 4, 5), (0, 1, 4, 5),
            (0, 1, 2, 5), (0, 1, 3, 4), (1, 2, 4, 5), (0, 2, 3, 5),
            (0, 1, 2, 3, 4, 5)]
A = 1.7159
S = 2.0 / 3.0
Q = 127.0                      # int8 quant scale for tanh in [-1, 1]

B, C, H, W = 256, 6, 142, 142
KH = KW = 5
OC = 16
OH, OW = H - 4, W - 4          # 138
NCORES = 8
B_LOC = B // NCORES            # 32
BF16 = ml_dtypes.bfloat16

# Column-parity formulation with pass-offset views: output (oc, j, p) at
# stream col (b, u) is y[b, oc, oh0 + j, 2u + p].  x is stored once as two
# column-parity classes; pass g reads the same tile at u-offset g, so its
# rows act as shift classes {2g, 2g+1} covering taps kw = 2g + cls - p.
T = 4                          # output rows per block
SPAN = 8                       # hh window
P2 = 2                         # column parities
U = OW // 2                    # 69 streamed positions
U2 = U + 2                     # 71 stored positions
M = OC * T * P2                # 128: m = oc*8 + j*2 + p
ROWB = 2 * C                   # 12 rows per hslot (cls, c)
KROWS = SPAN * ROWB            # 96
NST = 3                        # interleaved streams (ring prefetch depth)
SB = OH // NST                 # 46 output rows per stream
NBS = 12                       # blocks per stream (11 full + 1 two-row tail)
NSTEP = NST * NBS              # 36
NP = 3                         # passes (kw class pairs)
CB = 4                         # batches per psum chunk
NS = CB * U                    # 276 cols per matmul
PB = 512                       # psum bank stride (fp32)
SF = B_LOC * U                 # 2208 output cols per step
XCOL = B_LOC * U2              # 2272 stored cols per row
XROW = ROWB * XCOL             # 27264 elems per h in DRAM

_cache = {}


def _build():
    if 'nc' in _cache:
        return _cache['nc']
    f32 = mybir.dt.float32
    bf16 = mybir.dt.bfloat16
    i8 = mybir.dt.int8
    nc = bacc.Bacc("TRN2", target_bir_lowering=False, debug=False,
                   num_devices=NCORES)
    x_d = nc.dram_tensor("x", [H, XROW], bf16, kind="ExternalInput").ap()
    w_d = nc.dram_tensor("w", [KROWS, 9 * M], bf16, kind="ExternalInput").ap()
    b_d = nc.dram_tensor("b", [M, 1], f32, kind="ExternalInput").ap()
    y_d = nc.dram_tensor("y", [NSTEP, M, SF], i8, kind="ExternalOutput").ap()

    with tile.TileContext(nc) as tc:
        with tc.tile_pool(name="wpool", bufs=1) as wpool, \
             tc.tile_pool(name="tpool", bufs=3) as tpool, \
             tc.tile_pool(name="spool", bufs=3) as spool, \
             tc.tile_pool(name="pspool", bufs=1, space="PSUM") as pspool:

            xt = [wpool.tile([KROWS, XCOL], bf16, name=f"xt{k}")
                  for k in range(NST)]
            w_sb = wpool.tile([KROWS, 9 * M], bf16)
            b_sb = wpool.tile([M, 1], f32)

            # preload: stream 0's first batch-chunk first so matmuls start
            # early; streams 1/2 ride the scalar HWDGE queue set
            nc.gpsimd.dma_start(w_sb[:], w_d[:])
            nc.gpsimd.dma_start(b_sb[:], b_d[:])
            src0 = x_d[0:SPAN].rearrange("h (r n) -> (h r) n", r=ROWB)
            nc.sync.dma_start(xt[0][:, 0:CB * U2], src0[:, 0:CB * U2])
            nc.sync.dma_start(xt[0][:, CB * U2:], src0[:, CB * U2:])
            for k in (1, 2):
                src = x_d[SB * k:SB * k + SPAN]
                nc.gpsimd.dma_start(
                    xt[k][:], src.rearrange("h (r n) -> (h r) n", r=ROWB))

            for i in range(NBS):
                var = 2 if i == 11 else (i % 2)
                for k in range(NST):
                    step = i * NST + k
                    t_sb = tpool.tile([M, SF], bf16)
                    stage = spool.tile([M, SF], i8)
                    for half in range(2):
                        ps = pspool.tile([M, 4 * PB], f32, name=f"ps{half}",
                                         tag=f"ps{half}")
                        for k4 in range(4):
                            ch = half * 4 + k4
                            for g in range(NP):
                                rv = xt[k][:].rearrange(
                                    "k (b u) -> k b u", b=B_LOC)[
                                    :, ch * CB:(ch + 1) * CB, g:g + U]
                                nc.tensor.matmul(
                                    ps[:, k4 * PB:k4 * PB + NS],
                                    w_sb[:, (var * NP + g) * M:
                                         (var * NP + g + 1) * M],
                                    rv, start=(g == 0), stop=(g == NP - 1))
                        src_v = ps[:].rearrange("m (c n) -> m c n",
                                                n=PB)[:, :, 0:NS]
                        t_sl = t_sb[:, half * 4 * NS:(half + 1) * 4 * NS]
                        nc.scalar.activation(
                            t_sl.rearrange("m (c n) -> m c n", n=NS), src_v,
                            mybir.ActivationFunctionType.Tanh,
                            bias=b_sb[:], scale=S)
                        if half == 1 and i + 1 < NBS:
                            # ring refill for (k, i+1)
                            h0 = SB * k + 4 * (i + 1) + 4
                            hn = min(SB * k + 50, h0 + 4) - h0
                            sl = (4 * (i + 1) + 4) % 8
                            src = x_d[h0:h0 + hn]
                            nc.sync.dma_start(
                                xt[k][sl * ROWB:(sl + hn) * ROWB, :],
                                src.rearrange("h (r n) -> (h r) n", r=ROWB))
                        hs = slice(half * 4 * NS, (half + 1) * 4 * NS)
                        nc.vector.tensor_scalar_mul(stage[:, hs], t_sb[:, hs], Q)
                    yeng = nc.gpsimd if step % 2 == 0 else nc.scalar
                    yeng.dma_start(y_d[step], stage[:])
    nc.compile()
    _cache['nc'] = nc
    return nc


def _prep_weights(w3, b3, w4, b4, w6, b6):
    Wd = np.zeros((OC, C, KH, KW), np.float32)
    bias = np.zeros((OC,), np.float32)
    for i, idx in enumerate(C3_TABLE[:6]):
        Wd[i, list(idx)] = w3[i]
        bias[i] = b3[i]
    for i, idx in enumerate(C3_TABLE[6:15]):
        Wd[6 + i, list(idx)] = w4[i]
        bias[6 + i] = b4[i]
    Wd[15, list(C3_TABLE[15])] = w6[0]
    bias[15] = b6[0]

    def build(rot, jmax):
        w = np.zeros((NP, KROWS, M), np.float32)
        for g in range(NP):
            for slot in range(SPAN):
                hh = (slot - rot) % 8
                for j in range(jmax):
                    kh = hh - j
                    if not (0 <= kh < KH):
                        continue
                    for cls in range(2):
                        for p in range(P2):
                            kw = 2 * g + cls - p
                            if not (0 <= kw < KW):
                                continue
                            k0 = slot * ROWB + cls * C
                            w[g, k0:k0 + C,
                              np.arange(OC) * 8 + j * 2 + p] = Wd[:, :, kh, kw]
        return w

    wk = np.stack([build(0, 4), build(4, 4), build(4, 2)])  # [var, g, k, m]
    wflat = wk.transpose(2, 0, 1, 3).reshape(KROWS, 9 * M)
    bvec = np.repeat(S * bias, T * P2).reshape(M, 1).astype(np.float32)
    return wflat.astype(BF16), bvec


def _prep_x(x_shard):
    # [B_LOC, C, H, W] f32 -> [H, (cls c b u)] bf16 parity classes
    xc = np.empty((H, 2, C, B_LOC, U2), BF16)
    for cls in range(2):
        xc[:, cls] = x_shard[:, :, :, cls:cls + 2 * U2:2].transpose(2, 1, 0, 3)
    return np.ascontiguousarray(xc.reshape(H, XROW))


def _unpack_y(y_s):
    v = np.asarray(y_s).astype(np.float32)
    v = v.reshape(NBS, NST, OC, T, P2, B_LOC, U)    # i, k, oc, j, p, b, u
    out = np.empty((B_LOC, OC, OH, OW), np.float32)
    for i in range(NBS):
        jmax = 2 if i == 11 else 4
        for k in range(NST):
            oh0 = SB * k + 4 * i
            blk = v[i, k, :, :jmax]                 # oc, j, p, b, u
            out[:, :, oh0:oh0 + jmax, 0::2] = blk[:, :, 0].transpose(2, 0, 1, 3)
            out[:, :, oh0:oh0 + jmax, 1::2] = blk[:, :, 1].transpose(2, 0, 1, 3)
    return out


def kernel(x, w3, b3, w4, b4, w6, b6):
    nc = _build()
    w3, b3, w4, b4, w6, b6 = [np.asarray(a, dtype=np.float32)
                              for a in (w3, b3, w4, b4, w6, b6)]
    wk, bvec = _prep_weights(w3, b3, w4, b4, w6, b6)
    x = np.ascontiguousarray(np.asarray(x), dtype=np.float32)
    in_maps = [{"x": _prep_x(x[i * B_LOC:(i + 1) * B_LOC]), "w": wk, "b": bvec}
               for i in range(NCORES)]
    res = run_bass_kernel_spmd(nc, in_maps, list(range(NCORES)))
    out = np.concatenate([_unpack_y(res.results[i]["y"]) for i in range(NCORES)],
                         axis=0)
    out *= (A / Q)
    return np.ascontiguousarray(out)


# revision 18
# speedup vs baseline: 1.0346x; 1.0041x over previous
import sys
sys.path.insert(0, '/opt/trn_rl_repo')
import numpy as np
import ml_dtypes
import concourse.bass as bass
import concourse.bacc as bacc
import concourse.tile as tile
import concourse.mybir as mybir
from concourse.bass_utils import run_bass_kernel_spmd

C3_TABLE = [(0, 1, 2), (1, 2, 3), (2, 3, 4), (3, 4, 5), (0, 4, 5), (0, 1, 5),
            (0, 1, 2, 3), (1, 2, 3, 4), (2, 3, 4, 5), (0, 3, 4, 5), (0, 1, 4, 5),
            (0, 1, 2, 5), (0, 1, 3, 4), (1, 2, 4, 5), (0, 2, 3, 5),
            (0, 1, 2, 3, 4, 5)]
A = 1.7159
S = 2.0 / 3.0
Q = 127.0                      # int8 quant scale for tanh in [-1, 1]

B, C, H, W = 256, 6, 142, 142
KH = KW = 5
OC = 16
OH, OW = H - 4, W - 4          # 138
NCORES = 8
B_LOC = B // NCORES            # 32
BF16 = ml_dtypes.bfloat16

# Column-parity formulation with pass-offset views: output (oc, j, p) at
# stream col (b, u) is y[b, oc, oh0 + j, 2u + p].  x is stored once as two
# column-parity classes; pass g reads the same tile at u-offset g, so its
# rows act as shift classes {2g, 2g+1} covering taps kw = 2g + cls - p.
T = 4                          # output rows per block
SPAN = 8                       # hh window
P2 = 2                         # column parities
U = OW // 2                    # 69 streamed positions
U2 = U + 2                     # 71 stored positions
M = OC * T * P2                # 128: m = oc*8 + j*2 + p
ROWB = 2 * C                   # 12 rows per hslot (cls, c)
KROWS = SPAN * ROWB            # 96
NST = 3                        # interleaved streams (ring prefetch depth)
SB0 = [0, 48, 96]              # stream oh starts
SBL = [48, 48, 42]             # stream lengths (12, 12, 11 blocks)
NBS = 12                       # max blocks per stream
NSTEP = 35
NP = 3                         # passes (kw class pairs)
CB = 4                         # batches per psum chunk
NS = CB * U                    # 276 cols per matmul
PB = 512                       # psum bank stride (fp32)
SF = B_LOC * U                 # 2208 output cols per step
XCOL = B_LOC * U2              # 2272 stored cols per row
XROW = ROWB * XCOL             # 27264 elems per h in DRAM

_cache = {}


def _build():
    if 'nc' in _cache:
        return _cache['nc']
    f32 = mybir.dt.float32
    bf16 = mybir.dt.bfloat16
    i8 = mybir.dt.int8
    nc = bacc.Bacc("TRN2", target_bir_lowering=False, debug=False,
                   num_devices=NCORES)
    x_d = nc.dram_tensor("x", [H, XROW], bf16, kind="ExternalInput").ap()
    w_d = nc.dram_tensor("w", [KROWS, 9 * M], bf16, kind="ExternalInput").ap()
    b_d = nc.dram_tensor("b", [M, 1], f32, kind="ExternalInput").ap()
    y_d = nc.dram_tensor("y", [NSTEP, M, SF], i8, kind="ExternalOutput").ap()

    with tile.TileContext(nc) as tc:
        with tc.tile_pool(name="wpool", bufs=1) as wpool, \
             tc.tile_pool(name="tpool", bufs=3) as tpool, \
             tc.tile_pool(name="spool", bufs=3) as spool, \
             tc.tile_pool(name="pspool", bufs=1, space="PSUM") as pspool:

            xt = [wpool.tile([KROWS, XCOL], bf16, name=f"xt{k}")
                  for k in range(NST)]
            w_sb = wpool.tile([KROWS, 9 * M], bf16)
            b_sb = wpool.tile([M, 1], f32)

            # preload: stream 0's first batch-chunk first so matmuls start
            # early; streams 1/2 ride the scalar HWDGE queue set
            nc.gpsimd.dma_start(w_sb[:], w_d[:])
            nc.gpsimd.dma_start(b_sb[:], b_d[:])
            src0 = x_d[0:SPAN].rearrange("h (r n) -> (h r) n", r=ROWB)
            nc.sync.dma_start(xt[0][:, 0:CB * U2], src0[:, 0:CB * U2])
            nc.sync.dma_start(xt[0][:, CB * U2:], src0[:, CB * U2:])
            for k in (1, 2):
                src = x_d[SB0[k]:SB0[k] + SPAN]
                nc.gpsimd.dma_start(
                    xt[k][:], src.rearrange("h (r n) -> (h r) n", r=ROWB))

            step = -1
            for i in range(NBS):
                for k in range(NST):
                    if 4 * i >= SBL[k]:
                        continue
                    tail = (4 * i + 4 > SBL[k])
                    var = 2 if tail else (i % 2)
                    step += 1
                    t_sb = tpool.tile([M, SF], bf16)
                    stage = spool.tile([M, SF], i8)
                    for half in range(2):
                        ps = pspool.tile([M, 4 * PB], f32, name=f"ps{half}",
                                         tag=f"ps{half}")
                        for k4 in range(4):
                            ch = half * 4 + k4
                            for g in range(NP):
                                rv = xt[k][:].rearrange(
                                    "k (b u) -> k b u", b=B_LOC)[
                                    :, ch * CB:(ch + 1) * CB, g:g + U]
                                nc.tensor.matmul(
                                    ps[:, k4 * PB:k4 * PB + NS],
                                    w_sb[:, (var * NP + g) * M:
                                         (var * NP + g + 1) * M],
                                    rv, start=(g == 0), stop=(g == NP - 1))
                        src_v = ps[:].rearrange("m (c n) -> m c n",
                                                n=PB)[:, :, 0:NS]
                        t_sl = t_sb[:, half * 4 * NS:(half + 1) * 4 * NS]
                        nc.scalar.activation(
                            t_sl.rearrange("m (c n) -> m c n", n=NS), src_v,
                            mybir.ActivationFunctionType.Tanh,
                            bias=b_sb[:], scale=S)
                        if half == 1 and 4 * (i + 1) < SBL[k]:
                            # ring refill for (k, i+1)
                            h0 = SB0[k] + 4 * (i + 1) + 4
                            hn = min(SB0[k] + SBL[k] + 4, h0 + 4) - h0
                            sl = (4 * (i + 1) + 4) % 8
                            src = x_d[h0:h0 + hn]
                            nc.sync.dma_start(
                                xt[k][sl * ROWB:(sl + hn) * ROWB, :],
                                src.rearrange("h (r n) -> (h r) n", r=ROWB))
                        hs = slice(half * 4 * NS, (half + 1) * 4 * NS)
                        nc.vector.tensor_scalar_mul(stage[:, hs], t_sb[:, hs], Q)
                    yeng = nc.gpsimd if step % 2 == 0 else nc.scalar
                    yeng.dma_start(y_d[step], stage[:])
    nc.compile()
    _cache['nc'] = nc
    return nc


def _prep_weights(w3, b3, w4, b4, w6, b6):
    Wd = np.zeros((OC, C, KH, KW), np.float32)
    bias = np.zeros((OC,), np.float32)
    for i, idx in enumerate(C3_TABLE[:6]):
        Wd[i, list(idx)] = w3[i]
        bias[i] = b3[i]
    for i, idx in enumerate(C3_TABLE[6:15]):
        Wd[6 + i, list(idx)] = w4[i]
        bias[6 + i] = b4[i]
    Wd[15, list(C3_TABLE[15])] = w6[0]
    bias[15] = b6[0]

    def build(rot, jmax):
        w = np.zeros((NP, KROWS, M), np.float32)
        for g in range(NP):
            for slot in range(SPAN):
                hh = (slot - rot) % 8
                for j in range(jmax):
                    kh = hh - j
                    if not (0 <= kh < KH):
                        continue
                    for cls in range(2):
                        for p in range(P2):
                            kw = 2 * g + cls - p
                            if not (0 <= kw < KW):
                                continue
                            k0 = slot * ROWB + cls * C
                            w[g, k0:k0 + C,
                              np.arange(OC) * 8 + j * 2 + p] = Wd[:, :, kh, kw]
        return w

    wk = np.stack([build(0, 4), build(4, 4), build(0, 2)])  # [var, g, k, m]
    wflat = wk.transpose(2, 0, 1, 3).reshape(KROWS, 9 * M)
    bvec = np.repeat(S * bias, T * P2).reshape(M, 1).astype(np.float32)
    return wflat.astype(BF16), bvec


def _prep_x(x_shard):
    # [B_LOC, C, H, W] f32 -> [H, (cls c b u)] bf16 parity classes
    xc = np.empty((H, 2, C, B_LOC, U2), BF16)
    for cls in range(2):
        xc[:, cls] = x_shard[:, :, :, cls:cls + 2 * U2:2].transpose(2, 1, 0, 3)
    return np.ascontiguousarray(xc.reshape(H, XROW))


def _unpack_y(y_s):
    v = np.asarray(y_s).astype(np.float32)
    v = v.reshape(NSTEP, OC, T, P2, B_LOC, U)       # step, oc, j, p, b, u
    out = np.empty((B_LOC, OC, OH, OW), np.float32)
    step = -1
    for i in range(NBS):
        for k in range(NST):
            if 4 * i >= SBL[k]:
                continue
            step += 1
            jmax = min(4, SBL[k] - 4 * i)
            oh0 = SB0[k] + 4 * i
            blk = v[step, :, :jmax]                 # oc, j, p, b, u
            out[:, :, oh0:oh0 + jmax, 0::2] = blk[:, :, 0].transpose(2, 0, 1, 3)
            out[:, :, oh0:oh0 + jmax, 1::2] = blk[:, :, 1].transpose(2, 0, 1, 3)
    return out


def kernel(x, w3, b3, w4, b4, w6, b6):
    nc = _build()
    w3, b3, w4, b4, w6, b6 = [np.asarray(a, dtype=np.float32)
                              for a in (w3, b3, w4, b4, w6, b6)]
    wk, bvec = _prep_weights(w3, b3, w4, b4, w6, b6)
    x = np.ascontiguousarray(np.asarray(x), dtype=np.float32)
    in_maps = [{"x": _prep_x(x[i * B_LOC:(i + 1) * B_LOC]), "w": wk, "b": bvec}
               for i in range(NCORES)]
    res = run_bass_kernel_spmd(nc, in_maps, list(range(NCORES)))
    out = np.concatenate([_unpack_y(res.results[i]["y"]) for i in range(NCORES)],
                         axis=0)
    out *= (A / Q)
    return np.ascontiguousarray(out)


# revision 19
# speedup vs baseline: 1.0517x; 1.0165x over previous
import sys
sys.path.insert(0, '/opt/trn_rl_repo')
import numpy as np
import ml_dtypes
import concourse.bass as bass
import concourse.bacc as bacc
import concourse.tile as tile
import concourse.mybir as mybir
from concourse.bass_utils import run_bass_kernel_spmd

C3_TABLE = [(0, 1, 2), (1, 2, 3), (2, 3, 4), (3, 4, 5), (0, 4, 5), (0, 1, 5),
            (0, 1, 2, 3), (1, 2, 3, 4), (2, 3, 4, 5), (0, 3, 4, 5), (0, 1, 4, 5),
            (0, 1, 2, 5), (0, 1, 3, 4), (1, 2, 4, 5), (0, 2, 3, 5),
            (0, 1, 2, 3, 4, 5)]
A = 1.7159
S = 2.0 / 3.0
Q = 127.0                      # int8 quant scale for tanh in [-1, 1]

B, C, H, W = 256, 6, 142, 142
KH = KW = 5
OC = 16
OH, OW = H - 4, W - 4          # 138
NCORES = 8
B_LOC = B // NCORES            # 32
BF16 = ml_dtypes.bfloat16

# Column-parity formulation with pass-offset views: output (oc, j, p) at
# stream col (b, u) is y[b, oc, oh0 + j, 2u + p].  x is stored once as two
# column-parity classes; pass g reads the same tile at u-offset g, so its
# rows act as shift classes {2g, 2g+1} covering taps kw = 2g + cls - p.
T = 4                          # output rows per block
SPAN = 8                       # hh window
P2 = 2                         # column parities
U = OW // 2                    # 69 streamed positions
U2 = U + 2                     # 71 stored positions
M = OC * T * P2                # 128: m = oc*8 + j*2 + p
ROWB = 2 * C                   # 12 rows per hslot (cls, c)
KROWS = SPAN * ROWB            # 96
NST = 3                        # interleaved streams (ring prefetch depth)
SB0 = [0, 48, 96]              # stream oh starts
SBL = [48, 48, 42]             # stream lengths (12, 12, 11 blocks)
NBS = 12                       # max blocks per stream
NSTEP = 35
NP = 3                         # passes (kw class pairs)
CB = 4                         # batches per psum chunk
NS = CB * U                    # 276 cols per matmul
PB = 512                       # psum bank stride (fp32)
SF = B_LOC * U                 # 2208 output cols per step
XCOL = B_LOC * U2              # 2272 stored cols per row
XROW = ROWB * XCOL             # 27264 elems per h in DRAM

_cache = {}


def _build():
    if 'nc' in _cache:
        return _cache['nc']
    f32 = mybir.dt.float32
    bf16 = mybir.dt.bfloat16
    i8 = mybir.dt.int8
    nc = bacc.Bacc("TRN2", target_bir_lowering=False, debug=False,
                   num_devices=NCORES)
    x_d = nc.dram_tensor("x", [H, XROW], bf16, kind="ExternalInput").ap()
    w_d = nc.dram_tensor("w", [KROWS, 9 * M], bf16, kind="ExternalInput").ap()
    b_d = nc.dram_tensor("b", [M, 1], f32, kind="ExternalInput").ap()
    y_d = nc.dram_tensor("y", [NSTEP, M, SF], i8, kind="ExternalOutput").ap()

    with tile.TileContext(nc) as tc:
        with tc.tile_pool(name="wpool", bufs=1) as wpool, \
             tc.tile_pool(name="tpool", bufs=3) as tpool, \
             tc.tile_pool(name="spool", bufs=3) as spool, \
             tc.tile_pool(name="pspool", bufs=1, space="PSUM") as pspool:

            xt = [wpool.tile([KROWS, XCOL], bf16, name=f"xt{k}")
                  for k in range(NST)]
            w_sb = wpool.tile([KROWS, 9 * M], bf16)
            b_sb = wpool.tile([M, 1], f32)

            # preload: stream 0's first batch-chunk first so matmuls start
            # early; streams 1/2 ride the scalar HWDGE queue set
            nc.gpsimd.dma_start(w_sb[:], w_d[:])
            nc.gpsimd.dma_start(b_sb[:], b_d[:])
            src0 = x_d[0:SPAN].rearrange("h (r n) -> (h r) n", r=ROWB)
            nc.gpsimd.dma_start(xt[0][:, 0:CB * U2], src0[:, 0:CB * U2])
            nc.sync.dma_start(xt[0][:, CB * U2:3 * CB * U2],
                              src0[:, CB * U2:3 * CB * U2])
            nc.sync.dma_start(xt[0][:, 3 * CB * U2:], src0[:, 3 * CB * U2:])
            for k in (1, 2):
                src = x_d[SB0[k]:SB0[k] + SPAN]
                nc.gpsimd.dma_start(
                    xt[k][:], src.rearrange("h (r n) -> (h r) n", r=ROWB))

            step = -1
            for i in range(NBS):
                for k in range(NST):
                    if 4 * i >= SBL[k]:
                        continue
                    tail = (4 * i + 4 > SBL[k])
                    var = 2 if tail else (i % 2)
                    step += 1
                    t_sb = tpool.tile([M, SF], bf16)
                    stage = spool.tile([M, SF], i8)
                    for half in range(2):
                        ps = pspool.tile([M, 4 * PB], f32, name=f"ps{half}",
                                         tag=f"ps{half}")
                        for k4 in range(4):
                            ch = half * 4 + k4
                            for g in range(NP):
                                rv = xt[k][:].rearrange(
                                    "k (b u) -> k b u", b=B_LOC)[
                                    :, ch * CB:(ch + 1) * CB, g:g + U]
                                nc.tensor.matmul(
                                    ps[:, k4 * PB:k4 * PB + NS],
                                    w_sb[:, (var * NP + g) * M:
                                         (var * NP + g + 1) * M],
                                    rv, start=(g == 0), stop=(g == NP - 1))
                        src_v = ps[:].rearrange("m (c n) -> m c n",
                                                n=PB)[:, :, 0:NS]
                        t_sl = t_sb[:, half * 4 * NS:(half + 1) * 4 * NS]
                        nc.scalar.activation(
                            t_sl.rearrange("m (c n) -> m c n", n=NS), src_v,
                            mybir.ActivationFunctionType.Tanh,
                            bias=b_sb[:], scale=S)
                        if half == 1 and 4 * (i + 1) < SBL[k]:
                            # ring refill for (k, i+1)
                            h0 = SB0[k] + 4 * (i + 1) + 4
                            hn = min(SB0[k] + SBL[k] + 4, h0 + 4) - h0
                            sl = (4 * (i + 1) + 4) % 8
                            src = x_d[h0:h0 + hn]
                            nc.sync.dma_start(
                                xt[k][sl * ROWB:(sl + hn) * ROWB, :],
                                src.rearrange("h (r n) -> (h r) n", r=ROWB))
                        hs = slice(half * 4 * NS, (half + 1) * 4 * NS)
                        nc.vector.tensor_scalar_mul(stage[:, hs], t_sb[:, hs], Q)
                    yeng = nc.gpsimd if step % 2 == 0 else nc.scalar
                    yeng.dma_start(y_d[step], stage[:])
    nc.compile()
    _cache['nc'] = nc
    return nc


def _prep_weights(w3, b3, w4, b4, w6, b6):
    Wd = np.zeros((OC, C, KH, KW), np.float32)
    bias = np.zeros((OC,), np.float32)
    for i, idx in enumerate(C3_TABLE[:6]):
        Wd[i, list(idx)] = w3[i]
        bias[i] = b3[i]
    for i, idx in enumerate(C3_TABLE[6:15]):
        Wd[6 + i, list(idx)] = w4[i]
        bias[6 + i] = b4[i]
    Wd[15, list(C3_TABLE[15])] = w6[0]
    bias[15] = b6[0]

    def build(rot, jmax):
        w = np.zeros((NP, KROWS, M), np.float32)
        for g in range(NP):
            for slot in range(SPAN):
                hh = (slot - rot) % 8
                for j in range(jmax):
                    kh = hh - j
                    if not (0 <= kh < KH):
                        continue
                    for cls in range(2):
                        for p in range(P2):
                            kw = 2 * g + cls - p
                            if not (0 <= kw < KW):
                                continue
                            k0 = slot * ROWB + cls * C
                            w[g, k0:k0 + C,
                              np.arange(OC) * 8 + j * 2 + p] = Wd[:, :, kh, kw]
        return w

    wk = np.stack([build(0, 4), build(4, 4), build(0, 2)])  # [var, g, k, m]
    wflat = wk.transpose(2, 0, 1, 3).reshape(KROWS, 9 * M)
    bvec = np.repeat(S * bias, T * P2).reshape(M, 1).astype(np.float32)
    return wflat.astype(BF16), bvec


def _prep_x(x_shard):
    # [B_LOC, C, H, W] f32 -> [H, (cls c b u)] bf16 parity classes
    xc = np.empty((H, 2, C, B_LOC, U2), BF16)
    for cls in range(2):
        xc[:, cls] = x_shard[:, :, :, cls:cls + 2 * U2:2].transpose(2, 1, 0, 3)
    return np.ascontiguousarray(xc.reshape(H, XROW))


def _unpack_y(y_s):
    v = np.asarray(y_s).astype(np.float32)
    v = v.reshape(NSTEP, OC, T, P2, B_LOC, U)       # step, oc, j, p, b, u
    out = np.empty((B_LOC, OC, OH, OW), np.float32)
    step = -1
    for i in range(NBS):
        for k in range(NST):
            if 4 * i >= SBL[k]:
                continue
            step += 1
            jmax = min(4, SBL[k] - 4 * i)
            oh0 = SB0[k] + 4 * i
            blk = v[step, :, :jmax]                 # oc, j, p, b, u
            out[:, :, oh0:oh0 + jmax, 0::2] = blk[:, :, 0].transpose(2, 0, 1, 3)
            out[:, :, oh0:oh0 + jmax, 1::2] = blk[:, :, 1].transpose(2, 0, 1, 3)
    return out


def kernel(x, w3, b3, w4, b4, w6, b6):
    nc = _build()
    w3, b3, w4, b4, w6, b6 = [np.asarray(a, dtype=np.float32)
                              for a in (w3, b3, w4, b4, w6, b6)]
    wk, bvec = _prep_weights(w3, b3, w4, b4, w6, b6)
    x = np.ascontiguousarray(np.asarray(x), dtype=np.float32)
    in_maps = [{"x": _prep_x(x[i * B_LOC:(i + 1) * B_LOC]), "w": wk, "b": bvec}
               for i in range(NCORES)]
    res = run_bass_kernel_spmd(nc, in_maps, list(range(NCORES)))
    out = np.concatenate([_unpack_y(res.results[i]["y"]) for i in range(NCORES)],
                         axis=0)
    out *= (A / Q)
    return np.ascontiguousarray(out)


# revision 20
# speedup vs baseline: 1.0566x; 1.0047x over previous
import sys
sys.path.insert(0, '/opt/trn_rl_repo')
import numpy as np
import ml_dtypes
import concourse.bass as bass
import concourse.bacc as bacc
import concourse.tile as tile
import concourse.mybir as mybir
from concourse.bass_utils import run_bass_kernel_spmd

C3_TABLE = [(0, 1, 2), (1, 2, 3), (2, 3, 4), (3, 4, 5), (0, 4, 5), (0, 1, 5),
            (0, 1, 2, 3), (1, 2, 3, 4), (2, 3, 4, 5), (0, 3, 4, 5), (0, 1, 4, 5),
            (0, 1, 2, 5), (0, 1, 3, 4), (1, 2, 4, 5), (0, 2, 3, 5),
            (0, 1, 2, 3, 4, 5)]
A = 1.7159
S = 2.0 / 3.0
Q = 127.0                      # int8 quant scale for tanh in [-1, 1]

B, C, H, W = 256, 6, 142, 142
KH = KW = 5
OC = 16
OH, OW = H - 4, W - 4          # 138
NCORES = 8
B_LOC = B // NCORES            # 32
BF16 = ml_dtypes.bfloat16

# Column-parity formulation with pass-offset views: output (oc, j, p) at
# stream col (b, u) is y[b, oc, oh0 + j, 2u + p].  x is stored once as two
# column-parity classes; pass g reads the same tile at u-offset g, so its
# rows act as shift classes {2g, 2g+1} covering taps kw = 2g + cls - p.
T = 4                          # output rows per block
SPAN = 8                       # hh window
P2 = 2                         # column parities
U = OW // 2                    # 69 streamed positions
U2 = U + 2                     # 71 stored positions
M = OC * T * P2                # 128: m = oc*8 + j*2 + p
ROWB = 2 * C                   # 12 rows per hslot (cls, c)
KROWS = SPAN * ROWB            # 96
NST = 3                        # interleaved streams (ring prefetch depth)
SB0 = [0, 48, 96]              # stream oh starts
SBL = [48, 48, 42]             # stream lengths (12, 12, 11 blocks)
NBS = 12                       # max blocks per stream
NSTEP = 35
NP = 3                         # passes (kw class pairs)
CB = 4                         # batches per psum chunk
NS = CB * U                    # 276 cols per matmul
PB = 512                       # psum bank stride (fp32)
SF = B_LOC * U                 # 2208 output cols per step
XCOL = B_LOC * U2              # 2272 stored cols per row
XROW = ROWB * XCOL             # 27264 elems per h in DRAM

_cache = {}


def _build():
    if 'nc' in _cache:
        return _cache['nc']
    f32 = mybir.dt.float32
    bf16 = mybir.dt.bfloat16
    i8 = mybir.dt.int8
    nc = bacc.Bacc("TRN2", target_bir_lowering=False, debug=False,
                   num_devices=NCORES)
    x_d = nc.dram_tensor("x", [H, XROW], bf16, kind="ExternalInput").ap()
    w_d = nc.dram_tensor("w", [KROWS, 9 * M], bf16, kind="ExternalInput").ap()
    b_d = nc.dram_tensor("b", [M, 1], f32, kind="ExternalInput").ap()
    y_d = nc.dram_tensor("y", [NSTEP, M, SF], i8, kind="ExternalOutput").ap()

    with tile.TileContext(nc) as tc:
        with tc.tile_pool(name="wpool", bufs=1) as wpool, \
             tc.tile_pool(name="tpool", bufs=3) as tpool, \
             tc.tile_pool(name="spool", bufs=3) as spool, \
             tc.tile_pool(name="pspool", bufs=1, space="PSUM") as pspool:

            xt = [wpool.tile([KROWS, XCOL], bf16, name=f"xt{k}")
                  for k in range(NST)]
            w_sb = wpool.tile([KROWS, 9 * M], bf16)
            b_sb = wpool.tile([M, 1], f32)

            # preload: stream 0's first batch-chunk first so matmuls start
            # early; streams 1/2 ride the scalar HWDGE queue set
            nc.gpsimd.dma_start(w_sb[:], w_d[:])
            nc.gpsimd.dma_start(b_sb[:], b_d[:])
            src0 = x_d[0:SPAN].rearrange("h (r n) -> (h r) n", r=ROWB)
            nc.gpsimd.dma_start(xt[0][:, 0:CB * U2], src0[:, 0:CB * U2])
            nc.sync.dma_start(xt[0][:, CB * U2:3 * CB * U2],
                              src0[:, CB * U2:3 * CB * U2])
            nc.sync.dma_start(xt[0][:, 3 * CB * U2:], src0[:, 3 * CB * U2:])
            for k in (1, 2):
                src = x_d[SB0[k]:SB0[k] + SPAN]
                nc.gpsimd.dma_start(
                    xt[k][:], src.rearrange("h (r n) -> (h r) n", r=ROWB))

            step = -1
            for i in range(NBS):
                for k in range(NST):
                    if 4 * i >= SBL[k]:
                        continue
                    tail = (4 * i + 4 > SBL[k])
                    var = 2 if tail else (i % 2)
                    step += 1
                    t_sb = tpool.tile([M, SF], bf16)
                    stage = spool.tile([M, SF], i8)
                    for half in range(2):
                        ps = pspool.tile([M, 4 * PB], f32, name=f"ps{half}",
                                         tag=f"ps{half}")
                        for k4 in range(4):
                            ch = half * 4 + k4
                            for g in range(NP):
                                rv = xt[k][:].rearrange(
                                    "k (b u) -> k b u", b=B_LOC)[
                                    :, ch * CB:(ch + 1) * CB, g:g + U]
                                nc.tensor.matmul(
                                    ps[:, k4 * PB:k4 * PB + NS],
                                    w_sb[:, (var * NP + g) * M:
                                         (var * NP + g + 1) * M],
                                    rv, start=(g == 0), stop=(g == NP - 1))
                        src_v = ps[:].rearrange("m (c n) -> m c n",
                                                n=PB)[:, :, 0:NS]
                        t_sl = t_sb[:, half * 4 * NS:(half + 1) * 4 * NS]
                        nc.scalar.activation(
                            t_sl.rearrange("m (c n) -> m c n", n=NS), src_v,
                            mybir.ActivationFunctionType.Tanh,
                            bias=b_sb[:], scale=S)
                        if half == 1 and 4 * (i + 1) < SBL[k]:
                            # ring refill for (k, i+1)
                            h0 = SB0[k] + 4 * (i + 1) + 4
                            hn = min(SB0[k] + SBL[k] + 4, h0 + 4) - h0
                            sl = (4 * (i + 1) + 4) % 8
                            src = x_d[h0:h0 + hn]
                            nc.sync.dma_start(
                                xt[k][sl * ROWB:(sl + hn) * ROWB, :],
                                src.rearrange("h (r n) -> (h r) n", r=ROWB))
                        hs = slice(half * 4 * NS, (half + 1) * 4 * NS)
                        nc.vector.tensor_scalar_mul(stage[:, hs], t_sb[:, hs], Q)
                        if i == NBS - 1 and k == 1:
                            # final step: store per half so the last transfer
                            # is small and overlaps the drain
                            eng2 = nc.gpsimd if half == 0 else nc.scalar
                            eng2.dma_start(y_d[step, :, hs], stage[:, hs])
                    if not (i == NBS - 1 and k == 1):
                        yeng = nc.gpsimd if step % 2 == 0 else nc.scalar
                        yeng.dma_start(y_d[step], stage[:])
    nc.compile()
    _cache['nc'] = nc
    return nc


def _prep_weights(w3, b3, w4, b4, w6, b6):
    Wd = np.zeros((OC, C, KH, KW), np.float32)
    bias = np.zeros((OC,), np.float32)
    for i, idx in enumerate(C3_TABLE[:6]):
        Wd[i, list(idx)] = w3[i]
        bias[i] = b3[i]
    for i, idx in enumerate(C3_TABLE[6:15]):
        Wd[6 + i, list(idx)] = w4[i]
        bias[6 + i] = b4[i]
    Wd[15, list(C3_TABLE[15])] = w6[0]
    bias[15] = b6[0]

    def build(rot, jmax):
        w = np.zeros((NP, KROWS, M), np.float32)
        for g in range(NP):
            for slot in range(SPAN):
                hh = (slot - rot) % 8
                for j in range(jmax):
                    kh = hh - j
                    if not (0 <= kh < KH):
                        continue
                    for cls in range(2):
                        for p in range(P2):
                            kw = 2 * g + cls - p
                            if not (0 <= kw < KW):
                                continue
                            k0 = slot * ROWB + cls * C
                            w[g, k0:k0 + C,
                              np.arange(OC) * 8 + j * 2 + p] = Wd[:, :, kh, kw]
        return w

    wk = np.stack([build(0, 4), build(4, 4), build(0, 2)])  # [var, g, k, m]
    wflat = wk.transpose(2, 0, 1, 3).reshape(KROWS, 9 * M)
    bvec = np.repeat(S * bias, T * P2).reshape(M, 1).astype(np.float32)
    return wflat.astype(BF16), bvec


def _prep_x(x_shard):
    # [B_LOC, C, H, W] f32 -> [H, (cls c b u)] bf16 parity classes
    xc = np.empty((H, 2, C, B_LOC, U2), BF16)
    for cls in range(2):
        xc[:, cls] = x_shard[:, :, :, cls:cls + 2 * U2:2].transpose(2, 1, 0, 3)
    return np.ascontiguousarray(xc.reshape(H, XROW))


def _unpack_y(y_s):
    v = np.asarray(y_s).astype(np.float32)
    v = v.reshape(NSTEP, OC, T, P2, B_LOC, U)       # step, oc, j, p, b, u
    out = np.empty((B_LOC, OC, OH, OW), np.float32)
    step = -1
    for i in range(NBS):
        for k in range(NST):
            if 4 * i >= SBL[k]:
                continue
            step += 1
            jmax = min(4, SBL[k] - 4 * i)
            oh0 = SB0[k] + 4 * i
            blk = v[step, :, :jmax]                 # oc, j, p, b, u
            out[:, :, oh0:oh0 + jmax, 0::2] = blk[:, :, 0].transpose(2, 0, 1, 3)
            out[:, :, oh0:oh0 + jmax, 1::2] = blk[:, :, 1].transpose(2, 0, 1, 3)
    return out


def kernel(x, w3, b3, w4, b4, w6, b6):
    nc = _build()
    w3, b3, w4, b4, w6, b6 = [np.asarray(a, dtype=np.float32)
                              for a in (w3, b3, w4, b4, w6, b6)]
    wk, bvec = _prep_weights(w3, b3, w4, b4, w6, b6)
    x = np.ascontiguousarray(np.asarray(x), dtype=np.float32)
    in_maps = [{"x": _prep_x(x[i * B_LOC:(i + 1) * B_LOC]), "w": wk, "b": bvec}
               for i in range(NCORES)]
    res = run_bass_kernel_spmd(nc, in_maps, list(range(NCORES)))
    out = np.concatenate([_unpack_y(res.results[i]["y"]) for i in range(NCORES)],
                         axis=0)
    out *= (A / Q)
    return np.ascontiguousarray(out)


# revision 21
# speedup vs baseline: 1.0599x; 1.0032x over previous
import sys
sys.path.insert(0, '/opt/trn_rl_repo')
import numpy as np
import ml_dtypes
import concourse.bass as bass
import concourse.bacc as bacc
import concourse.tile as tile
import concourse.mybir as mybir
from concourse.bass_utils import run_bass_kernel_spmd

C3_TABLE = [(0, 1, 2), (1, 2, 3), (2, 3, 4), (3, 4, 5), (0, 4, 5), (0, 1, 5),
            (0, 1, 2, 3), (1, 2, 3, 4), (2, 3, 4, 5), (0, 3, 4, 5), (0, 1, 4, 5),
            (0, 1, 2, 5), (0, 1, 3, 4), (1, 2, 4, 5), (0, 2, 3, 5),
            (0, 1, 2, 3, 4, 5)]
A = 1.7159
S = 2.0 / 3.0
Q = 127.0                      # int8 quant scale for tanh in [-1, 1]

B, C, H, W = 256, 6, 142, 142
KH = KW = 5
OC = 16
OH, OW = H - 4, W - 4          # 138
NCORES = 8
B_LOC = B // NCORES            # 32
BF16 = ml_dtypes.bfloat16

# Column-parity formulation with pass-offset views: output (oc, j, p) at
# stream col (b, u) is y[b, oc, oh0 + j, 2u + p].  x is stored once as two
# column-parity classes; pass g reads the same tile at u-offset g, so its
# rows act as shift classes {2g, 2g+1} covering taps kw = 2g + cls - p.
T = 4                          # output rows per block
SPAN = 8                       # hh window
P2 = 2                         # column parities
U = OW // 2                    # 69 streamed positions
U2 = U + 2                     # 71 stored positions
M = OC * T * P2                # 128: m = oc*8 + j*2 + p
ROWB = 2 * C                   # 12 rows per hslot (cls, c)
KROWS = SPAN * ROWB            # 96
NST = 3                        # interleaved streams (ring prefetch depth)
SB0 = [0, 48, 96]              # stream oh starts
SBL = [48, 48, 42]             # stream lengths (12, 12, 11 blocks)
NBS = 12                       # max blocks per stream
NSTEP = 35
NP = 3                         # passes (kw class pairs)
CB = 4                         # batches per psum chunk
NS = CB * U                    # 276 cols per matmul
PB = 512                       # psum bank stride (fp32)
SF = B_LOC * U                 # 2208 output cols per step
XCOL = B_LOC * U2              # 2272 stored cols per row
XROW = ROWB * XCOL             # 27264 elems per h in DRAM

_cache = {}


def _build():
    if 'nc' in _cache:
        return _cache['nc']
    f32 = mybir.dt.float32
    bf16 = mybir.dt.bfloat16
    i8 = mybir.dt.int8
    nc = bacc.Bacc("TRN2", target_bir_lowering=False, debug=False,
                   num_devices=NCORES)
    x_d = nc.dram_tensor("x", [H, XROW], bf16, kind="ExternalInput").ap()
    w_d = nc.dram_tensor("w", [KROWS, 9 * M], bf16, kind="ExternalInput").ap()
    b_d = nc.dram_tensor("b", [M, 1], f32, kind="ExternalInput").ap()
    y_d = nc.dram_tensor("y", [NSTEP, M, SF], i8, kind="ExternalOutput").ap()

    with tile.TileContext(nc) as tc:
        with tc.tile_pool(name="wpool", bufs=1) as wpool, \
             tc.tile_pool(name="tpool", bufs=3) as tpool, \
             tc.tile_pool(name="spool", bufs=3) as spool, \
             tc.tile_pool(name="pspool", bufs=1, space="PSUM") as pspool:

            xt = [wpool.tile([KROWS, XCOL], bf16, name=f"xt{k}")
                  for k in range(NST)]
            w_sb = wpool.tile([KROWS, 9 * M], bf16)
            b_sb = wpool.tile([M, 1], f32)

            # p-state warm-up: run the PE on zeroed SBUF while the preloads
            # are in flight so the clock ramp finishes before real matmuls
            wt = wpool.tile([KROWS, 512], bf16)
            nc.vector.memset(wt[:], 0)
            ps_w = pspool.tile([M, 4 * PB], f32, name="ps0", tag="ps0")
            for wi in range(6):
                nc.tensor.matmul(ps_w[:, 0:512], wt[:, 0:M], wt[:],
                                 start=(wi == 0), stop=(wi == 5))

            # preload: stream 0's first batch-chunk first so matmuls start
            # early; streams 1/2 ride the scalar HWDGE queue set
            nc.gpsimd.dma_start(w_sb[:], w_d[:])
            nc.gpsimd.dma_start(b_sb[:], b_d[:])
            src0 = x_d[0:SPAN].rearrange("h (r n) -> (h r) n", r=ROWB)
            nc.gpsimd.dma_start(xt[0][:, 0:CB * U2], src0[:, 0:CB * U2])
            nc.sync.dma_start(xt[0][:, CB * U2:3 * CB * U2],
                              src0[:, CB * U2:3 * CB * U2])
            nc.sync.dma_start(xt[0][:, 3 * CB * U2:], src0[:, 3 * CB * U2:])
            for k in (1, 2):
                src = x_d[SB0[k]:SB0[k] + SPAN]
                nc.gpsimd.dma_start(
                    xt[k][:], src.rearrange("h (r n) -> (h r) n", r=ROWB))

            step = -1
            for i in range(NBS):
                for k in range(NST):
                    if 4 * i >= SBL[k]:
                        continue
                    tail = (4 * i + 4 > SBL[k])
                    var = 2 if tail else (i % 2)
                    step += 1
                    t_sb = tpool.tile([M, SF], bf16)
                    stage = spool.tile([M, SF], i8)
                    for half in range(2):
                        ps = pspool.tile([M, 4 * PB], f32, name=f"ps{half}",
                                         tag=f"ps{half}")
                        for k4 in range(4):
                            ch = half * 4 + k4
                            for g in range(NP):
                                rv = xt[k][:].rearrange(
                                    "k (b u) -> k b u", b=B_LOC)[
                                    :, ch * CB:(ch + 1) * CB, g:g + U]
                                nc.tensor.matmul(
                                    ps[:, k4 * PB:k4 * PB + NS],
                                    w_sb[:, (var * NP + g) * M:
                                         (var * NP + g + 1) * M],
                                    rv, start=(g == 0), stop=(g == NP - 1))
                        src_v = ps[:].rearrange("m (c n) -> m c n",
                                                n=PB)[:, :, 0:NS]
                        t_sl = t_sb[:, half * 4 * NS:(half + 1) * 4 * NS]
                        nc.scalar.activation(
                            t_sl.rearrange("m (c n) -> m c n", n=NS), src_v,
                            mybir.ActivationFunctionType.Tanh,
                            bias=b_sb[:], scale=S)
                        if half == 1 and 4 * (i + 1) < SBL[k]:
                            # ring refill for (k, i+1)
                            h0 = SB0[k] + 4 * (i + 1) + 4
                            hn = min(SB0[k] + SBL[k] + 4, h0 + 4) - h0
                            sl = (4 * (i + 1) + 4) % 8
                            src = x_d[h0:h0 + hn]
                            nc.sync.dma_start(
                                xt[k][sl * ROWB:(sl + hn) * ROWB, :],
                                src.rearrange("h (r n) -> (h r) n", r=ROWB))
                        hs = slice(half * 4 * NS, (half + 1) * 4 * NS)
                        nc.vector.tensor_scalar_mul(stage[:, hs], t_sb[:, hs], Q)
                        if i == NBS - 1 and k == 1:
                            # final step: store per half so the last transfer
                            # is small and overlaps the drain
                            eng2 = nc.gpsimd if half == 0 else nc.scalar
                            eng2.dma_start(y_d[step, :, hs], stage[:, hs])
                    if not (i == NBS - 1 and k == 1):
                        yeng = nc.gpsimd if step % 3 == 0 else nc.scalar
                        yeng.dma_start(y_d[step], stage[:])
    nc.compile()
    _cache['nc'] = nc
    return nc


def _prep_weights(w3, b3, w4, b4, w6, b6):
    Wd = np.zeros((OC, C, KH, KW), np.float32)
    bias = np.zeros((OC,), np.float32)
    for i, idx in enumerate(C3_TABLE[:6]):
        Wd[i, list(idx)] = w3[i]
        bias[i] = b3[i]
    for i, idx in enumerate(C3_TABLE[6:15]):
        Wd[6 + i, list(idx)] = w4[i]
        bias[6 + i] = b4[i]
    Wd[15, list(C3_TABLE[15])] = w6[0]
    bias[15] = b6[0]

    def build(rot, jmax):
        w = np.zeros((NP, KROWS, M), np.float32)
        for g in range(NP):
            for slot in range(SPAN):
                hh = (slot - rot) % 8
                for j in range(jmax):
                    kh = hh - j
                    if not (0 <= kh < KH):
                        continue
                    for cls in range(2):
                        for p in range(P2):
                            kw = 2 * g + cls - p
                            if not (0 <= kw < KW):
                                continue
                            k0 = slot * ROWB + cls * C
                            w[g, k0:k0 + C,
                              np.arange(OC) * 8 + j * 2 + p] = Wd[:, :, kh, kw]
        return w

    wk = np.stack([build(0, 4), build(4, 4), build(0, 2)])  # [var, g, k, m]
    wflat = wk.transpose(2, 0, 1, 3).reshape(KROWS, 9 * M)
    bvec = np.repeat(S * bias, T * P2).reshape(M, 1).astype(np.float32)
    return wflat.astype(BF16), bvec


def _prep_x(x_shard):
    # [B_LOC, C, H, W] f32 -> [H, (cls c b u)] bf16 parity classes
    xc = np.empty((H, 2, C, B_LOC, U2), BF16)
    for cls in range(2):
        xc[:, cls] = x_shard[:, :, :, cls:cls + 2 * U2:2].transpose(2, 1, 0, 3)
    return np.ascontiguousarray(xc.reshape(H, XROW))


def _unpack_y(y_s):
    v = np.asarray(y_s).astype(np.float32)
    v = v.reshape(NSTEP, OC, T, P2, B_LOC, U)       # step, oc, j, p, b, u
    out = np.empty((B_LOC, OC, OH, OW), np.float32)
    step = -1
    for i in range(NBS):
        for k in range(NST):
            if 4 * i >= SBL[k]:
                continue
            step += 1
            jmax = min(4, SBL[k] - 4 * i)
            oh0 = SB0[k] + 4 * i
            blk = v[step, :, :jmax]                 # oc, j, p, b, u
            out[:, :, oh0:oh0 + jmax, 0::2] = blk[:, :, 0].transpose(2, 0, 1, 3)
            out[:, :, oh0:oh0 + jmax, 1::2] = blk[:, :, 1].transpose(2, 0, 1, 3)
    return out


def kernel(x, w3, b3, w4, b4, w6, b6):
    nc = _build()
    w3, b3, w4, b4, w6, b6 = [np.asarray(a, dtype=np.float32)
                              for a in (w3, b3, w4, b4, w6, b6)]
    wk, bvec = _prep_weights(w3, b3, w4, b4, w6, b6)
    x = np.ascontiguousarray(np.asarray(x), dtype=np.float32)
    in_maps = [{"x": _prep_x(x[i * B_LOC:(i + 1) * B_LOC]), "w": wk, "b": bvec}
               for i in range(NCORES)]
    res = run_bass_kernel_spmd(nc, in_maps, list(range(NCORES)))
    out = np.concatenate([_unpack_y(res.results[i]["y"]) for i in range(NCORES)],
                         axis=0)
    out *= (A / Q)
    return np.ascontiguousarray(out)
